# revision 11
# baseline (speedup 1.0000x reference)
"""Capsule dynamic-routing kernel for Trainium2 (Bass/Tile), 8 NeuronCores.

Sharding: data-parallel over batch (B=64 -> 8 batches/core, grouped in 4
pairs of 2). W (64x256) is tiny and folded into per-iteration stationary
operands; no collectives are needed (pure SPMD).

The reference computes
    u_hat = u @ W                      # (N, 256), col c = k*16+d
    b=0; for i in 3: c = softmax_k(b); s[k,:] = sum_n c[k,n]*u_hat[n,kblk];
         out = squash(s); b += <out, u_hat>
u_hat is (B,N,256) = 512 MiB and never fits on chip.  We never materialize
it.  Since b_i = <sum_{j<i} out_j, u_hat>, with O = accumulated outputs and
Obd its (256,16) block-diagonal expansion:
    b_i[k,n] = <Wo[:,k], u[n,:]>   where Wo = W @ Obd   (64x16, tiny)
    s[k,d]   = sum_e G[k,e] W[e,k*16+d],  G[k,e] = sum_n c[k,n] u[n,e]
so each routing iteration only streams u (SBUF-resident, bf16) through the
PE array.

SBUF residents per core (bf16):
    ut[p] (128=2bx64e, N)      e-on-partitions ("transposed") for the b-pass
    un[p] (128=n-in-chunk, N)  n-on-partitions, chunk-major free, for G
b-pass per 128-n chunk:  bbT(128n, 32=2bx16k) = ut_chunk.T @ WoPair
softmax: free-dim (over k) ops at full 128-lane occupancy, no transposes
G-pass per chunk:        GT(128=2bx64e, 32) += un_chunk.T @ C_chunk
finalize per batch:      S(16,256) = G_b.T.T @ W ; mask diag blocks; squash.
"""

import numpy as np
from contextlib import ExitStack

import ml_dtypes

import concourse.bass as bass
import concourse.bacc as bacc
import concourse.tile as tile
import concourse.mybir as mybir
from concourse.bass_utils import run_bass_kernel_spmd

dt = mybir.dt
AFT = mybir.ActivationFunctionType
AXT = mybir.AxisListType

B, N_FULL, D = 64, 8192, 64
K, DCAP, KD = 16, 16, 256
NCORES = 8
NB = 8            # batches per core
NP = 4            # batch pairs per core
ROUTINGS = 3
EPS = 1e-7
CHUNK = 128       # n per contraction chunk
SUP = 8           # chunks per softmax super-chunk
SUBCOLS = 2048    # free columns per resident DMA sub-tile

U_DT = dt.bfloat16
U_NP = ml_dtypes.bfloat16


def build_program(n=N_FULL, reps=1):
    assert n % CHUNK == 0
    nch = n // CHUNK
    sup = min(SUP, nch)
    assert nch % sup == 0
    nsup = nch // sup
    subcols = min(SUBCOLS, n)
    nsub = n // subcols
    f32 = dt.float32

    nc = bacc.Bacc("TRN2", target_bir_lowering=False, debug=False)

    ut_d = nc.dram_tensor("ut", [NP, 128, n], U_DT, kind="ExternalInput").ap()
    un_d = nc.dram_tensor("un", [NP, 128, n], U_DT, kind="ExternalInput").ap()
    wt_d = nc.dram_tensor("wt", [2, 128, D], U_DT, kind="ExternalInput").ap()
    wsb_d = nc.dram_tensor("wsb", [128, KD], U_DT, kind="ExternalInput").ap()
    mask_d = nc.dram_tensor("mask", [128, KD], f32, kind="ExternalInput").ap()
    ident_d = nc.dram_tensor("ident", [128, 128], f32, kind="ExternalInput").ap()
    out_d = nc.dram_tensor("out", [128, KD], f32, kind="ExternalOutput").ap()

    with tile.TileContext(nc) as tc, ExitStack() as ctx:
        consts = ctx.enter_context(tc.tile_pool(name="consts", bufs=1))
        resident = ctx.enter_context(tc.tile_pool(name="resident", bufs=1))
        work = ctx.enter_context(tc.tile_pool(name="work", bufs=1))
        c_pool = ctx.enter_context(tc.tile_pool(name="cpool", bufs=3))
        e_pool = ctx.enter_context(tc.tile_pool(name="epool", bufs=2))
        z_pool = ctx.enter_context(tc.tile_pool(name="zpool", bufs=4))
        ps_bb = ctx.enter_context(tc.tile_pool(name="psbb", bufs=2, space="PSUM"))
        ps_gt = ctx.enter_context(tc.tile_pool(name="psgt", bufs=1, space="PSUM"))

        # ---- constants ----
        wt_t = consts.tile([128, 2 * D], U_DT, tag="wt", name="wt")        # W.T halves
        for h in range(2):
            nc.sync.dma_start(out=wt_t[:, h * D:(h + 1) * D], in_=wt_d[h])
        wsb_t = consts.tile([128, KD], U_DT, tag="wsb", name="wsb")         # W stacked x2
        nc.sync.dma_start(out=wsb_t[:, :], in_=wsb_d[:, :])
        mask_t = consts.tile([128, KD], f32, tag="mask", name="mask")
        nc.sync.dma_start(out=mask_t[:, :], in_=mask_d[:, :])
        ident_t = consts.tile([128, 128], f32, tag="ident", name="ident")
        nc.sync.dma_start(out=ident_t[:, :], in_=ident_d[:, :])
        cu_t = consts.tile([128, 32], U_DT, tag="cu", name="cu")           # uniform c=1/16
        nc.vector.memset(cu_t[:, :], 1.0 / K)
        eps_t = consts.tile([128, 1], f32, tag="eps", name="eps")
        nc.vector.memset(eps_t[:, :], EPS)

        # ---- resident input copies ----
        un_t = [[resident.tile([128, subcols], U_DT, tag=f"un{p}_{q}", name=f"un{p}_{q}")
                 for q in range(nsub)] for p in range(NP)]
        ut_t = [[resident.tile([128, subcols], U_DT, tag=f"ut{p}_{q}", name=f"ut{p}_{q}")
                 for q in range(nsub)] for p in range(NP)]
        cpc = subcols // CHUNK  # chunks per sub-tile

        def un_chunk(p, j):
            return un_t[p][j // cpc][:, (j % cpc) * CHUNK:(j % cpc + 1) * CHUNK]

        def ut_chunk(p, j):
            return ut_t[p][j // cpc][:, (j % cpc) * CHUNK:(j % cpc + 1) * CHUNK]

        # ---- persistent work tiles ----
        o_acc = work.tile([128, KD], f32, tag="oacc", name="oacc")      # masked output accum
        s_all = work.tile([128, KD], f32, tag="sall", name="sall")
        sm = work.tile([128, KD], f32, tag="sm", name="sm")
        sq = work.tile([128, KD], f32, tag="sq", name="sq")
        o_fin = work.tile([128, KD], f32, tag="ofin", name="ofin")
        t1_sb = work.tile([128, 128], U_DT, tag="t1", name="t1")      # Obd halves
        t2_sb = work.tile([128, 128], U_DT, tag="t2", name="t2")
        wop = [work.tile([128, 32], U_DT, tag=f"wop{p}", name=f"wop{p}") for p in range(NP)]
        gt_sb = [work.tile([128, 32], U_DT, tag=f"gts{p}", name=f"gts{p}") for p in range(NP)]
        s2 = work.tile([128, 1], f32, tag="s2", name="s2")
        sc_a = work.tile([128, 1], f32, tag="sca", name="sca")
        sc_b = work.tile([128, 1], f32, tag="scb", name="scb")
        sc_c = work.tile([128, 1], f32, tag="scc", name="scc")
        sc_d = work.tile([128, 1], f32, tag="scd", name="scd")
        sc_e = work.tile([128, 1], f32, tag="sce", name="sce")

        gt_tiles = [ps_gt.tile([128, 32], f32, tag=f"gt{p}", name=f"gt{p}",
                       padded_shape=[128, 512]) for p in range(NP)]

        # cross-batch blocks of gt_sb / wop stay zero for the whole kernel
        for p in range(NP):
            nc.vector.memset(gt_sb[p][0:64, 16:32], 0.0)
            nc.vector.memset(gt_sb[p][64:128, 0:16], 0.0)
            nc.vector.memset(wop[p][0:64, 16:32], 0.0)
            nc.vector.memset(wop[p][64:128, 0:16], 0.0)

        def routing_pass(it):
            """b-pass (if it>0) + softmax + G-pass, accumulating gt_tiles."""
            for p in range(NP):
                for s in range(nsup):
                    if it == 0:
                        def c_src(rel):
                            return cu_t[:, :]
                    else:
                        bb = ps_bb.tile([128, sup * 32], f32, tag="bb", name="bb",
                                        padded_shape=[128, 512])
                        for rel in range(sup):
                            j = s * sup + rel
                            nc.tensor.matmul(
                                bb[:, rel * 32:(rel + 1) * 32],
                                lhsT=ut_chunk(p, j), rhs=wop[p][:, :],
                                start=(rel == 0), stop=(rel == sup - 1))
                        e_t = e_pool.tile([128, sup * 32], f32, tag="e", name="e")
                        nc.scalar.activation(e_t[:, :], bb[:, :], AFT.Exp)
                        z_t = z_pool.tile([128, sup * 2], f32, tag="z", name="z")
                        nc.vector.reduce_sum(
                            z_t[:, :].rearrange("p (a b) -> p a b", b=2),
                            e_t[:, :].rearrange("p (a b c) -> p a b c", b=2, c=K),
                            axis=AXT.X)
                        zr_t = z_pool.tile([128, sup * 2], f32, tag="zr", name="zr")
                        nc.vector.reciprocal(zr_t[:, :], z_t[:, :])
                        c_t = c_pool.tile([128, sup * 32], U_DT, tag="c", name="c")
                        nc.vector.tensor_mul(
                            c_t[:, :].rearrange("p (a b c) -> p a b c", b=2, c=K),
                            e_t[:, :].rearrange("p (a b c) -> p a b c", b=2, c=K),
                            zr_t[:, :].rearrange("p (a b) -> p a b", b=2)
                                .broadcast_to([128, sup, 2, K]))

                        def c_src(rel, c_t=c_t):
                            return c_t[:, rel * 32:(rel + 1) * 32]
                    for rel in range(sup):
                        j = s * sup + rel
                        nc.tensor.matmul(
                            gt_tiles[p][:, :],
                            lhsT=un_chunk(p, j), rhs=c_src(rel),
                            start=(j == 0), stop=(j == nch - 1))

        def finalize(it):
            """gt -> s_all -> mask -> squash -> (o_acc | o_fin); update Wo."""
            for p in range(NP):
                # keep only the in-batch diagonal blocks of GT-pair; the
                # cross-batch blocks are garbage and contract as zero
                nc.vector.tensor_copy(gt_sb[p][0:64, 0:16],
                                      gt_tiles[p][0:64, 0:16])
                nc.vector.tensor_copy(gt_sb[p][64:128, 16:32],
                                      gt_tiles[p][64:128, 16:32])
            for p in range(NP):
                sf = ps_bb.tile([32, KD], f32, tag="bb", name="sf",
                                padded_shape=[32, 512])
                nc.tensor.matmul(sf[:, :], lhsT=gt_sb[p][:, :],
                                 rhs=wsb_t[:, :], start=True, stop=True)
                nc.vector.tensor_copy(s_all[32 * p:32 * p + 32, :], sf[:, :])
            nc.vector.tensor_mul(sm[:, :], s_all[:, :], mask_t[:, :])
            # squash: scale = s2/(1+s2)/sqrt(s2+EPS), s2 = sum_d sm^2 (row sum)
            nc.scalar.activation(sq[:, :], sm[:, :], AFT.Square,
                                 accum_out=s2[:, :])
            nc.vector.tensor_scalar_add(sc_a[:, :], s2[:, :], 1.0)
            nc.vector.reciprocal(sc_b[:, :], sc_a[:, :])
            nc.scalar.activation(sc_c[:, :], s2[:, :], AFT.Sqrt,
                                 bias=eps_t[:, :])
            nc.vector.reciprocal(sc_d[:, :], sc_c[:, :])
            nc.vector.tensor_mul(sc_e[:, :], sc_b[:, :], sc_d[:, :])
            nc.vector.tensor_mul(sc_e[:, :], sc_e[:, :], s2[:, :])
            tgt = o_fin if it == ROUTINGS - 1 else o_acc
            if it == 1:
                nc.vector.tensor_scalar_mul(o_fin[:, :], sm[:, :], sc_e[:, :])
                nc.vector.tensor_add(o_acc[:, :], o_acc[:, :], o_fin[:, :])
            else:
                nc.vector.tensor_scalar_mul(tgt[:, :], sm[:, :], sc_e[:, :])
            if it == ROUTINGS - 1:
                nc.sync.dma_start(out=out_d[:, :], in_=o_fin[:, :])
                return
            # Obd_b (256,16 block-diag of O_b) as columns of o_acc.T halves
            tps = []
            for h, t_sb in ((0, t1_sb), (1, t2_sb)):
                tp = ps_bb.tile([128, 128], f32, tag="bb", name="tp",
                                padded_shape=[128, 512])
                nc.tensor.transpose(tp[:, :], o_acc[:, h * 128:(h + 1) * 128],
                                    ident_t[:, :])
                nc.vector.tensor_copy(t_sb[:, :], tp[:, :])
                tps.append(tp)
            # Wo_b = W @ Obd_b, accumulated over the two 128-row halves of W.T
            wo = ps_bb.tile([64, NB * K], f32, tag="bb", name="wo",
                            padded_shape=[64, 512])
            for h2 in range(2):
                for b in range(NB):
                    nc.tensor.matmul(
                        wo[:, b * K:(b + 1) * K],
                        lhsT=wt_t[:, h2 * D:(h2 + 1) * D],
                        rhs=(t1_sb, t2_sb)[h2][:, b * K:(b + 1) * K],
                        start=(h2 == 0 and b == 0),
                        stop=(h2 == 1 and b == NB - 1))
            for b in range(NB):
                p, h = b // 2, b % 2
                nc.vector.tensor_copy(
                    wop[p][64 * h:64 * h + 64, 16 * h:16 * h + 16],
                    wo[:, b * K:(b + 1) * K])

        for rep in range(reps):
            for p in range(NP):
                for q in range(nsub):
                    nc.sync.dma_start(
                        out=un_t[p][q][:, :],
                        in_=un_d[p, :, q * subcols:(q + 1) * subcols])
            for p in range(NP):
                for q in range(nsub):
                    nc.sync.dma_start(
                        out=ut_t[p][q][:, :],
                        in_=ut_d[p, :, q * subcols:(q + 1) * subcols])
            for it in range(ROUTINGS):
                routing_pass(it)
                finalize(it)
            if rep < reps - 1:
                tc.strict_bb_all_engine_barrier()

    nc.compile()
    return nc


def host_inputs(u_shard, W):
    """Per-core DRAM inputs from an (8, N, 64) f32 batch shard + W (64, 256)."""
    n = u_shard.shape[1]
    ut = np.ascontiguousarray(
        u_shard.reshape(NP, 2, n, D).transpose(0, 1, 3, 2).reshape(NP, 128, n)
    ).astype(U_NP)
    un = np.ascontiguousarray(
        u_shard.reshape(NP, 2, n // CHUNK, CHUNK, D)
        .transpose(0, 3, 2, 1, 4).reshape(NP, 128, n)
    ).astype(U_NP)
    return {"ut": ut, "un": un}


def host_consts(W):
    Wf = np.asarray(W, np.float32)
    wt = np.ascontiguousarray(Wf.T.reshape(2, 128, D)).astype(U_NP)
    wsb = np.ascontiguousarray(np.concatenate([Wf, Wf], 0)).astype(U_NP)
    base = np.kron(np.eye(K, dtype=np.float32), np.ones((1, DCAP), np.float32))
    mask = np.ascontiguousarray(np.tile(base, (NB, 1)))
    ident = np.eye(128, dtype=np.float32)
    return {"wt": wt, "wsb": wsb, "mask": mask, "ident": ident}


def extract_output(res_out):
    """(128, 256) masked f32 -> (8, 16, 16) squashed capsule outputs."""
    ar = np.arange(K)
    return res_out.reshape(NB, K, K, DCAP)[:, ar, ar, :]


_PROG_CACHE = {}


def _get_prog(n=N_FULL, reps=1):
    key = (n, reps)
    if key not in _PROG_CACHE:
        _PROG_CACHE[key] = build_program(n, reps)
    return _PROG_CACHE[key]


def kernel(u_vecs, W):
    u = np.ascontiguousarray(np.asarray(u_vecs, np.float32))
    assert u.shape == (B, N_FULL, D)
    nc = _get_prog()
    consts = host_consts(W)
    in_maps = [dict(consts, **host_inputs(u[c * NB:(c + 1) * NB], W))
               for c in range(NCORES)]
    res = run_bass_kernel_spmd(nc, in_maps, core_ids=list(range(NCORES)))
    return np.concatenate(
        [extract_output(res.results[c]["out"]) for c in range(NCORES)], axis=0
    ).astype(np.float32)


# revision 24
# speedup vs baseline: 1.8452x; 1.8452x over previous
"""Capsule dynamic-routing kernel for Trainium2 (Bass/Tile), 8 NeuronCores.

Sharding: data-parallel over batch (B=64 -> 8 batches/core, grouped in 4
pairs of 2). W (64x256) is tiny and folded into per-iteration stationary
operands; no collectives are needed (pure SPMD).

The reference computes
    u_hat = u @ W                      # (N, 256), col c = k*16+d
    b=0; for i in 3: c = softmax_k(b); s[k,:] = sum_n c[k,n]*u_hat[n,kblk];
         out = squash(s); b += <out, u_hat>
u_hat is (B,N,256) = 512 MiB and never fits on chip.  We never materialize
it.  Since b_i = <sum_{j<i} out_j, u_hat>, with O = accumulated outputs and
Obd its (256,16) block-diagonal expansion:
    b_i[k,n] = <Wo[:,k], u[n,:]>   where Wo = W @ Obd   (64x16, tiny)
    s[k,d]   = sum_e G[k,e] W[e,k*16+d],  G[k,e] = sum_n c[k,n] u[n,e]
so each routing iteration only streams u (SBUF-resident, bf16) through the
PE array.

SBUF residents per core (bf16):
    ut[p] (128=2bx64e, N)      e-on-partitions ("transposed") for the b-pass
    un[p] (128=n-in-chunk, N)  n-on-partitions, chunk-major free, for G
b-pass per 128-n chunk:  bbT(128n, 32=2bx16k) = ut_chunk.T @ WoPair
softmax: free-dim (over k) ops at full 128-lane occupancy, no transposes
G-pass per chunk:        GT(128=2bx64e, 32) += un_chunk.T @ C_chunk
finalize per batch:      S(16,256) = G_b.T.T @ W ; mask diag blocks; squash.
"""

import numpy as np
from contextlib import ExitStack

import ml_dtypes

import concourse.bass as bass
import concourse.bacc as bacc
import concourse.tile as tile
import concourse.mybir as mybir
from concourse.bass_utils import run_bass_kernel_spmd

dt = mybir.dt
AFT = mybir.ActivationFunctionType
AXT = mybir.AxisListType
ALU = mybir.AluOpType

B, N_FULL, D = 64, 8192, 64
K, DCAP, KD = 16, 16, 256
NCORES = 8
NB = 8            # batches per core
NP = 4            # batch pairs per core
ROUTINGS = 3
EPS = 1e-7
CHUNK = 128       # n per contraction chunk
SUP = 16          # chunks per softmax super-chunk
SUBCOLS = 2048    # free columns per resident DMA sub-tile

U_DT = dt.bfloat16
U_NP = ml_dtypes.bfloat16


def build_program(n=N_FULL, reps=1, ablate=()):
    assert n % CHUNK == 0
    nch = n // CHUNK
    sup = min(SUP, nch)
    assert nch % sup == 0
    nsup = nch // sup
    subcols = min(SUBCOLS, n)
    nsub = n // subcols
    f32 = dt.float32

    nc = bacc.Bacc("TRN2", target_bir_lowering=False, debug=False)

    ut_d = nc.dram_tensor("ut", [NP, 128, n], U_DT, kind="ExternalInput").ap()
    un_d = nc.dram_tensor("un", [NP, 128, n], U_DT, kind="ExternalInput").ap()
    wt_d = nc.dram_tensor("wt", [2, 128, D], U_DT, kind="ExternalInput").ap()
    wsb_d = nc.dram_tensor("wsb", [128, KD], U_DT, kind="ExternalInput").ap()
    mask_d = nc.dram_tensor("mask", [128, KD], f32, kind="ExternalInput").ap()
    ident_d = nc.dram_tensor("ident", [128, 128], f32, kind="ExternalInput").ap()
    out_d = nc.dram_tensor("out", [128, KD], f32, kind="ExternalOutput").ap()

    with tile.TileContext(nc) as tc, ExitStack() as ctx:
        consts = ctx.enter_context(tc.tile_pool(name="consts", bufs=1))
        resident = ctx.enter_context(tc.tile_pool(name="resident", bufs=1))
        work = ctx.enter_context(tc.tile_pool(name="work", bufs=1))
        c_pool = ctx.enter_context(tc.tile_pool(name="cpool", bufs=5))
        e_pool = ctx.enter_context(tc.tile_pool(name="epool", bufs=5))
        z_pool = ctx.enter_context(tc.tile_pool(name="zpool", bufs=8))
        ps_bb = ctx.enter_context(tc.tile_pool(name="psbb", bufs=3, space="PSUM"))
        ps_gt = ctx.enter_context(tc.tile_pool(name="psgt", bufs=1, space="PSUM"))

        # ---- constants ----
        wt_t = consts.tile([128, 2 * D], U_DT, tag="wt", name="wt")        # W.T halves
        for h in range(2):
            nc.sync.dma_start(out=wt_t[:, h * D:(h + 1) * D], in_=wt_d[h])
        wsb_t = consts.tile([128, KD], U_DT, tag="wsb", name="wsb")         # W stacked x2
        nc.sync.dma_start(out=wsb_t[:, :], in_=wsb_d[:, :])
        mask_t = consts.tile([128, KD], f32, tag="mask", name="mask")
        nc.sync.dma_start(out=mask_t[:, :], in_=mask_d[:, :])
        ident_t = consts.tile([128, 128], f32, tag="ident", name="ident")
        nc.sync.dma_start(out=ident_t[:, :], in_=ident_d[:, :])
        cu_t = consts.tile([128, 32], U_DT, tag="cu", name="cu")           # uniform c=1/16
        nc.vector.memset(cu_t[:, :], 1.0 / K)
        eps_t = consts.tile([128, 1], f32, tag="eps", name="eps")
        nc.vector.memset(eps_t[:, :], EPS)

        # ---- resident input copies ----
        un_t = [[resident.tile([128, subcols], U_DT, tag=f"un{p}_{q}", name=f"un{p}_{q}")
                 for q in range(nsub)] for p in range(NP)]
        ut_t = [[resident.tile([128, subcols], U_DT, tag=f"ut{p}_{q}", name=f"ut{p}_{q}")
                 for q in range(nsub)] for p in range(NP)]
        cpc = subcols // CHUNK  # chunks per sub-tile

        def un_chunk(p, j):
            return un_t[p][j // cpc][:, (j % cpc) * CHUNK:(j % cpc + 1) * CHUNK]

        def ut_chunk(p, j):
            return ut_t[p][j // cpc][:, (j % cpc) * CHUNK:(j % cpc + 1) * CHUNK]

        # ---- persistent work tiles ----
        o_acc = work.tile([128, KD], f32, tag="oacc", name="oacc")      # masked output accum
        s_all = work.tile([128, KD], f32, tag="sall", name="sall")
        sm = work.tile([128, KD], f32, tag="sm", name="sm")
        sq = work.tile([128, KD], f32, tag="sq", name="sq")
        o_fin = work.tile([128, KD], f32, tag="ofin", name="ofin")
        t1_sb = work.tile([128, 128], U_DT, tag="t1", name="t1")      # Obd halves
        t2_sb = work.tile([128, 128], U_DT, tag="t2", name="t2")
        wop = [work.tile([128, 32], U_DT, tag=f"wop{p}", name=f"wop{p}") for p in range(NP)]
        gt_sb = [work.tile([128, 32], U_DT, tag=f"gts{p}", name=f"gts{p}") for p in range(NP)]
        s2 = work.tile([128, 1], f32, tag="s2", name="s2")
        rs_a = [work.tile([128, 8], f32, tag=f"rsa{p}", name=f"rsa{p}")
                for p in range(NP)]
        rs_b = [work.tile([128, 1], f32, tag=f"rsb{p}", name=f"rsb{p}")
                for p in range(NP)]
        sc_a = work.tile([128, 1], f32, tag="sca", name="sca")
        sc_b = work.tile([128, 1], f32, tag="scb", name="scb")
        sc_c = work.tile([128, 1], f32, tag="scc", name="scc")
        sc_d = work.tile([128, 1], f32, tag="scd", name="scd")
        sc_e = work.tile([128, 1], f32, tag="sce", name="sce")

        gt_tiles = [ps_gt.tile([128, 32], f32, tag=f"gt{p}", name=f"gt{p}",
                       padded_shape=[128, 512]) for p in range(NP)]

        # cross-batch blocks of gt_sb / wop stay zero for the whole kernel
        for p in range(NP):
            nc.vector.memset(gt_sb[p][0:64, 16:32], 0.0)
            nc.vector.memset(gt_sb[p][64:128, 0:16], 0.0)
            nc.vector.memset(wop[p][0:64, 16:32], 0.0)
            nc.vector.memset(wop[p][64:128, 0:16], 0.0)

        def routing_pass(it):
            """b-pass (if it>0) + softmax + G-pass, accumulating gt_tiles."""
            for p in range(NP):
                for s in range(nsup):
                    if it == 0 or "nobb" in ablate:
                        def c_src(rel):
                            return cu_t[:, :]
                    else:
                        bb = ps_bb.tile([128, sup * 32], f32, tag="bb", name="bb",
                                        padded_shape=[128, 512])
                        for rel in range(sup):
                            j = s * sup + rel
                            nc.tensor.matmul(
                                bb[:, rel * 32:(rel + 1) * 32],
                                lhsT=ut_chunk(p, j), rhs=wop[p][:, :],
                                start=(rel == 0), stop=(rel == sup - 1))
                        e_t = e_pool.tile([128, sup * 32], f32, tag="e", name="e")
                        nc.scalar.activation(e_t[:, :], bb[:, :], AFT.Exp)
                        z_t = z_pool.tile([128, sup * 2], f32, tag="z", name="z")
                        nc.vector.reduce_sum(
                            z_t[:, :].rearrange("p (a b) -> p a b", b=2),
                            e_t[:, :].rearrange("p (a b c) -> p a b c", b=2, c=K),
                            axis=AXT.X)
                        zr_t = z_pool.tile([128, sup * 2], f32, tag="zr", name="zr")
                        nc.vector.reciprocal(zr_t[:, :], z_t[:, :])
                        c_t = c_pool.tile([128, sup * 32], U_DT, tag="c", name="c")
                        nc.vector.tensor_mul(
                            c_t[:, :].rearrange("p (a b c) -> p a b c", b=2, c=K),
                            e_t[:, :].rearrange("p (a b c) -> p a b c", b=2, c=K),
                            zr_t[:, :].rearrange("p (a b) -> p a b", b=2)
                                .broadcast_to([128, sup, 2, K]))

                        def c_src(rel, c_t=c_t):
                            return c_t[:, rel * 32:(rel + 1) * 32]
                    for rel in range(sup):
                        j = s * sup + rel
                        nc.tensor.matmul(
                            gt_tiles[p][:, :],
                            lhsT=un_chunk(p, j), rhs=c_src(rel),
                            start=(j == 0), stop=(j == nch - 1))

        def finalize(it):
            """gt -> s_all -> mask -> squash -> (o_acc | o_fin); update Wo."""
            for p in range(NP):
                # keep only the in-batch diagonal blocks of GT-pair;
                # cross-batch blocks are garbage and contract as zero
                nc.vector.tensor_copy(gt_sb[p][0:64, 0:16],
                                      gt_tiles[p][0:64, 0:16])
                nc.vector.tensor_copy(gt_sb[p][64:128, 16:32],
                                      gt_tiles[p][64:128, 16:32])
            for p in range(NP):
                sf = ps_bb.tile([32, KD], f32, tag="bb", name="sf",
                                padded_shape=[32, 512])
                nc.tensor.matmul(sf[:, :], lhsT=gt_sb[p][:, :],
                                 rhs=wsb_t[:, :], start=True, stop=True)
                # fused PSUM->SBUF copy + diagonal-block mask
                nc.vector.tensor_mul(sm[32 * p:32 * p + 32, :], sf[:, :],
                                     mask_t[32 * p:32 * p + 32, :])
            # squash: scale = s2/(1+s2)/sqrt(s2+EPS), s2 = sum_d sm^2 (row sum)
            nc.scalar.activation(sq[:, :], sm[:, :], AFT.Square,
                                 accum_out=s2[:, :])
            nc.vector.tensor_scalar_add(sc_a[:, :], s2[:, :], 1.0)
            nc.vector.reciprocal(sc_b[:, :], sc_a[:, :])
            nc.scalar.activation(sc_c[:, :], s2[:, :], AFT.Sqrt,
                                 bias=eps_t[:, :])
            nc.vector.reciprocal(sc_d[:, :], sc_c[:, :])
            nc.vector.tensor_mul(sc_e[:, :], sc_b[:, :], sc_d[:, :])
            nc.vector.tensor_mul(sc_e[:, :], sc_e[:, :], s2[:, :])
            tgt = o_fin if it == ROUTINGS - 1 else o_acc
            if it == 1:
                nc.vector.tensor_scalar_mul(o_fin[:, :], sm[:, :], sc_e[:, :])
                nc.vector.tensor_add(o_acc[:, :], o_acc[:, :], o_fin[:, :])
            else:
                nc.vector.tensor_scalar_mul(tgt[:, :], sm[:, :], sc_e[:, :])
            if it == ROUTINGS - 1:
                nc.sync.dma_start(out=out_d[:, :], in_=o_fin[:, :])
                return
            # Obd_b (256,16 block-diag of O_b) as columns of o_acc.T halves
            tps = []
            for h, t_sb in ((0, t1_sb), (1, t2_sb)):
                tp = ps_bb.tile([128, 128], f32, tag="bb", name="tp",
                                padded_shape=[128, 512])
                nc.tensor.transpose(tp[:, :], o_acc[:, h * 128:(h + 1) * 128],
                                    ident_t[:, :])
                nc.vector.tensor_copy(t_sb[:, :], tp[:, :])
                tps.append(tp)
            # Wo_b = W @ Obd_b, accumulated over the two 128-row halves of W.T
            wo = ps_bb.tile([64, NB * K], f32, tag="bb", name="wo",
                            padded_shape=[64, 512])
            for h2 in range(2):
                for b in range(NB):
                    nc.tensor.matmul(
                        wo[:, b * K:(b + 1) * K],
                        lhsT=wt_t[:, h2 * D:(h2 + 1) * D],
                        rhs=(t1_sb, t2_sb)[h2][:, b * K:(b + 1) * K],
                        start=(h2 == 0 and b == 0),
                        stop=(h2 == 1 and b == NB - 1))
            for b in range(NB):
                p, h = b // 2, b % 2
                nc.vector.tensor_copy(
                    wop[p][64 * h:64 * h + 64, 16 * h:16 * h + 16],
                    wo[:, b * K:(b + 1) * K])

        for rep in range(reps):
            if "nodma" not in ablate:
                for p in range(NP):
                    for q in range(nsub):
                        nc.sync.dma_start(
                            out=un_t[p][q][:, :],
                            in_=un_d[p, :, q * subcols:(q + 1) * subcols])
                for p in range(NP):
                    for q in range(nsub):
                        nc.sync.dma_start(
                            out=ut_t[p][q][:, :],
                            in_=ut_d[p, :, q * subcols:(q + 1) * subcols])
            elif rep == 0:
                for p in range(NP):
                    for q in range(nsub):
                        nc.vector.memset(un_t[p][q][:, 0:2], 0.0)
                        nc.vector.memset(ut_t[p][q][:, 0:2], 0.0)
            if "nocompute" not in ablate:
                for it in range(ROUTINGS):
                    routing_pass(it)
                    finalize(it)
            else:
                nc.vector.memset(o_fin[:, :], 0.0)
                nc.sync.dma_start(out=out_d[:, :], in_=o_fin[:, :])
            if rep < reps - 1:
                tc.strict_bb_all_engine_barrier()

    nc.compile()
    return nc


def host_inputs(u_shard, W):
    """Per-core DRAM inputs from an (8, N, 64) f32 batch shard + W (64, 256)."""
    n = u_shard.shape[1]
    ut = np.ascontiguousarray(
        u_shard.reshape(NP, 2, n, D).transpose(0, 1, 3, 2).reshape(NP, 128, n)
    ).astype(U_NP)
    un = np.ascontiguousarray(
        u_shard.reshape(NP, 2, n // CHUNK, CHUNK, D)
        .transpose(0, 3, 2, 1, 4).reshape(NP, 128, n)
    ).astype(U_NP)
    return {"ut": ut, "un": un}


def host_consts(W):
    Wf = np.asarray(W, np.float32)
    wt = np.ascontiguousarray(Wf.T.reshape(2, 128, D)).astype(U_NP)
    wsb = np.ascontiguousarray(np.concatenate([Wf, Wf], 0)).astype(U_NP)
    base = np.kron(np.eye(K, dtype=np.float32), np.ones((1, DCAP), np.float32))
    mask = np.ascontiguousarray(np.tile(base, (NB, 1)))
    ident = np.eye(128, dtype=np.float32)
    return {"wt": wt, "wsb": wsb, "mask": mask, "ident": ident}


def extract_output(res_out):
    """(128, 256) masked f32 -> (8, 16, 16) squashed capsule outputs."""
    ar = np.arange(K)
    return res_out.reshape(NB, K, K, DCAP)[:, ar, ar, :]


_PROG_CACHE = {}


def _get_prog(n=N_FULL, reps=1):
    key = (n, reps)
    if key not in _PROG_CACHE:
        _PROG_CACHE[key] = build_program(n, reps)
    return _PROG_CACHE[key]


def kernel(u_vecs, W):
    u = np.ascontiguousarray(np.asarray(u_vecs, np.float32))
    assert u.shape == (B, N_FULL, D)
    nc = _get_prog()
    consts = host_consts(W)
    in_maps = [dict(consts, **host_inputs(u[c * NB:(c + 1) * NB], W))
               for c in range(NCORES)]
    res = run_bass_kernel_spmd(nc, in_maps, core_ids=list(range(NCORES)))
    return np.concatenate(
        [extract_output(res.results[c]["out"]) for c in range(NCORES)], axis=0
    ).astype(np.float32)


# revision 29
# speedup vs baseline: 1.9039x; 1.0318x over previous
"""Capsule dynamic-routing kernel for Trainium2 (Bass/Tile), 8 NeuronCores.

Sharding: data-parallel over batch (B=64 -> 8 batches/core, grouped in 4
pairs of 2). W (64x256) is tiny and folded into per-iteration stationary
operands; no collectives are needed (pure SPMD).

The reference computes
    u_hat = u @ W                      # (N, 256), col c = k*16+d
    b=0; for i in 3: c = softmax_k(b); s[k,:] = sum_n c[k,n]*u_hat[n,kblk];
         out = squash(s); b += <out, u_hat>
u_hat is (B,N,256) = 512 MiB and never fits on chip.  We never materialize
it.  Since b_i = <sum_{j<i} out_j, u_hat>, with O = accumulated outputs and
Obd its (256,16) block-diagonal expansion:
    b_i[k,n] = <Wo[:,k], u[n,:]>   where Wo = W @ Obd   (64x16, tiny)
    s[k,d]   = sum_e G[k,e] W[e,k*16+d],  G[k,e] = sum_n c[k,n] u[n,e]
so each routing iteration only streams u (SBUF-resident, bf16) through the
PE array.

SBUF residents per core (bf16):
    ut[p] (128=2bx64e, N)      e-on-partitions ("transposed") for the b-pass
    un[p] (128=n-in-chunk, N)  n-on-partitions, chunk-major free, for G
b-pass per 128-n chunk:  bbT(128n, 32=2bx16k) = ut_chunk.T @ WoPair
softmax: free-dim (over k) ops at full 128-lane occupancy, no transposes
G-pass per chunk:        GT(128=2bx64e, 32) += un_chunk.T @ C_chunk
finalize per batch:      S(16,256) = G_b.T.T @ W ; mask diag blocks; squash.
"""

import numpy as np
from contextlib import ExitStack

import ml_dtypes

import concourse.bass as bass
import concourse.bacc as bacc
import concourse.tile as tile
import concourse.mybir as mybir
from concourse.bass_utils import run_bass_kernel_spmd

dt = mybir.dt
AFT = mybir.ActivationFunctionType
AXT = mybir.AxisListType
ALU = mybir.AluOpType

B, N_FULL, D = 64, 8192, 64
K, DCAP, KD = 16, 16, 256
NCORES = 8
NB = 8            # batches per core
NP = 4            # batch pairs per core
ROUTINGS = 3
EPS = 1e-7
CHUNK = 128       # n per contraction chunk
SUP = 16          # chunks per softmax super-chunk
SUBCOLS = 2048    # free columns per resident DMA sub-tile

U_DT = dt.bfloat16
U_NP = ml_dtypes.bfloat16


def build_program(n=N_FULL, reps=1, ablate=()):
    assert n % CHUNK == 0
    nch = n // CHUNK
    sup = min(SUP, nch)
    assert nch % sup == 0
    nsup = nch // sup
    subcols = min(SUBCOLS, n)
    nsub = n // subcols
    f32 = dt.float32

    nc = bacc.Bacc("TRN2", target_bir_lowering=False, debug=False)

    ut_d = nc.dram_tensor("ut", [NP, 128, n], U_DT, kind="ExternalInput").ap()
    un_d = nc.dram_tensor("un", [n // 512, 128, 2048],
                          U_DT, kind="ExternalInput").ap()
    wt_d = nc.dram_tensor("wt", [2, 128, D], U_DT, kind="ExternalInput").ap()
    wsb_d = nc.dram_tensor("wsb", [128, KD], U_DT, kind="ExternalInput").ap()
    mask_d = nc.dram_tensor("mask", [128, KD], f32, kind="ExternalInput").ap()
    ident_d = nc.dram_tensor("ident", [128, 128], f32, kind="ExternalInput").ap()
    out_d = nc.dram_tensor("out", [128, KD], f32, kind="ExternalOutput").ap()

    with tile.TileContext(nc) as tc, ExitStack() as ctx:
        consts = ctx.enter_context(tc.tile_pool(name="consts", bufs=1))
        resident = ctx.enter_context(tc.tile_pool(name="resident", bufs=1))
        work = ctx.enter_context(tc.tile_pool(name="work", bufs=1))
        c_pool = ctx.enter_context(tc.tile_pool(name="cpool", bufs=5))
        e_pool = ctx.enter_context(tc.tile_pool(name="epool", bufs=5))
        z_pool = ctx.enter_context(tc.tile_pool(name="zpool", bufs=8))
        ps_bb = ctx.enter_context(tc.tile_pool(name="psbb", bufs=3, space="PSUM"))
        ps_gt = ctx.enter_context(tc.tile_pool(name="psgt", bufs=1, space="PSUM"))

        # ---- constants ----
        wt_t = consts.tile([128, 2 * D], U_DT, tag="wt", name="wt")        # W.T halves
        for h in range(2):
            nc.sync.dma_start(out=wt_t[:, h * D:(h + 1) * D], in_=wt_d[h])
        wsb_t = consts.tile([128, KD], U_DT, tag="wsb", name="wsb")         # W stacked x2
        nc.sync.dma_start(out=wsb_t[:, :], in_=wsb_d[:, :])
        mask_t = consts.tile([128, KD], f32, tag="mask", name="mask")
        nc.sync.dma_start(out=mask_t[:, :], in_=mask_d[:, :])
        ident_t = consts.tile([128, 128], f32, tag="ident", name="ident")
        nc.sync.dma_start(out=ident_t[:, :], in_=ident_d[:, :])
        cu_t = consts.tile([128, 32], U_DT, tag="cu", name="cu")           # uniform c=1/16
        nc.vector.memset(cu_t[:, :], 1.0 / K)
        eps_t = consts.tile([128, 1], f32, tag="eps", name="eps")
        nc.vector.memset(eps_t[:, :], EPS)

        # ---- resident input copies ----
        nsubq = nch // 4  # un subtiles: 4 chunks x (4 pairs x 128) each
        un_t = [resident.tile([128, 2048], U_DT, tag=f"un{q}", name=f"un{q}")
                for q in range(nsubq)]
        ut_t = [[resident.tile([128, subcols], U_DT, tag=f"ut{p}_{q}", name=f"ut{p}_{q}")
                 for q in range(nsub)] for p in range(NP)]
        cpc = subcols // CHUNK  # chunks per sub-tile

        def un_chunk(p, j):
            base = (j % 4) * 512 + p * CHUNK
            return un_t[j // 4][:, base:base + CHUNK]

        def un_quad(j):
            return un_t[j // 4][:, (j % 4) * 512:(j % 4) * 512 + 512]

        def ut_chunk(p, j):
            return ut_t[p][j // cpc][:, (j % cpc) * CHUNK:(j % cpc + 1) * CHUNK]

        # ---- persistent work tiles ----
        o_acc = work.tile([128, KD], f32, tag="oacc", name="oacc")      # masked output accum
        s_all = work.tile([128, KD], f32, tag="sall", name="sall")
        sm = work.tile([128, KD], f32, tag="sm", name="sm")
        sq = work.tile([128, KD], f32, tag="sq", name="sq")
        o_fin = work.tile([128, KD], f32, tag="ofin", name="ofin")
        t1_sb = work.tile([128, 128], U_DT, tag="t1", name="t1")      # Obd halves
        t2_sb = work.tile([128, 128], U_DT, tag="t2", name="t2")
        wop = [work.tile([128, 32], U_DT, tag=f"wop{p}", name=f"wop{p}") for p in range(NP)]
        gt_sb = [work.tile([128, 32], U_DT, tag=f"gts{p}", name=f"gts{p}") for p in range(NP)]
        s2 = work.tile([128, 1], f32, tag="s2", name="s2")
        g0_sb = work.tile([32, 512], f32, tag="g0sb", name="g0sb")
        sc_a = work.tile([128, 1], f32, tag="sca", name="sca")
        sc_b = work.tile([128, 1], f32, tag="scb", name="scb")
        sc_c = work.tile([128, 1], f32, tag="scc", name="scc")
        sc_d = work.tile([128, 1], f32, tag="scd", name="scd")
        sc_e = work.tile([128, 1], f32, tag="sce", name="sce")

        gt_tiles = [ps_gt.tile([128, 32], f32, tag=f"gt{p}", name=f"gt{p}",
                       padded_shape=[128, 512]) for p in range(NP)]

        # cross-batch blocks of gt_sb / wop stay zero for the whole kernel
        for p in range(NP):
            nc.vector.memset(gt_sb[p][0:64, 16:32], 0.0)
            nc.vector.memset(gt_sb[p][64:128, 0:16], 0.0)
            nc.vector.memset(wop[p][0:64, 16:32], 0.0)
            nc.vector.memset(wop[p][64:128, 0:16], 0.0)

        def routing_pass(it):
            """b-pass (if it>0) + softmax + G-pass, accumulating gt_tiles."""
            for p in range(NP):
                for s in range(nsup):
                    if it == 0 or "nobb" in ablate:
                        def c_src(rel):
                            return cu_t[:, :]
                    else:
                        bb = ps_bb.tile([128, sup * 32], f32, tag="bb", name="bb",
                                        padded_shape=[128, 512])
                        for rel in range(sup):
                            j = s * sup + rel
                            nc.tensor.matmul(
                                bb[:, rel * 32:(rel + 1) * 32],
                                lhsT=ut_chunk(p, j), rhs=wop[p][:, :],
                                start=(rel == 0), stop=(rel == sup - 1))
                        e_t = e_pool.tile([128, sup * 32], f32, tag="e", name="e")
                        nc.scalar.activation(e_t[:, :], bb[:, :], AFT.Exp)
                        z_t = z_pool.tile([128, sup * 2], f32, tag="z", name="z")
                        nc.vector.reduce_sum(
                            z_t[:, :].rearrange("p (a b) -> p a b", b=2),
                            e_t[:, :].rearrange("p (a b c) -> p a b c", b=2, c=K),
                            axis=AXT.X)
                        zr_t = z_pool.tile([128, sup * 2], f32, tag="zr", name="zr")
                        nc.vector.reciprocal(zr_t[:, :], z_t[:, :])
                        c_t = c_pool.tile([128, sup * 32], U_DT, tag="c", name="c")
                        nc.vector.tensor_mul(
                            c_t[:, :].rearrange("p (a b c) -> p a b c", b=2, c=K),
                            e_t[:, :].rearrange("p (a b c) -> p a b c", b=2, c=K),
                            zr_t[:, :].rearrange("p (a b) -> p a b", b=2)
                                .broadcast_to([128, sup, 2, K]))

                        def c_src(rel, c_t=c_t):
                            return c_t[:, rel * 32:(rel + 1) * 32]
                    for rel in range(sup):
                        j = s * sup + rel
                        nc.tensor.matmul(
                            gt_tiles[p][:, :],
                            lhsT=un_chunk(p, j), rhs=c_src(rel),
                            start=(j == 0), stop=(j == nch - 1))

        def finalize(it):
            """gt -> s_all -> mask -> squash -> (o_acc | o_fin); update Wo."""
            for p in range(NP):
                # keep only the in-batch diagonal blocks of GT-pair;
                # cross-batch blocks are garbage and contract as zero
                nc.vector.tensor_copy(gt_sb[p][0:64, 0:16],
                                      gt_tiles[p][0:64, 0:16])
                nc.vector.tensor_copy(gt_sb[p][64:128, 16:32],
                                      gt_tiles[p][64:128, 16:32])
            for p in range(NP):
                sf = ps_bb.tile([32, KD], f32, tag="bb", name="sf",
                                padded_shape=[32, 512])
                nc.tensor.matmul(sf[:, :], lhsT=gt_sb[p][:, :],
                                 rhs=wsb_t[:, :], start=True, stop=True)
                # fused PSUM->SBUF copy + diagonal-block mask
                nc.vector.tensor_mul(sm[32 * p:32 * p + 32, :], sf[:, :],
                                     mask_t[32 * p:32 * p + 32, :])
            # squash: scale = s2/(1+s2)/sqrt(s2+EPS), s2 = sum_d sm^2 (row sum)
            nc.scalar.activation(sq[:, :], sm[:, :], AFT.Square,
                                 accum_out=s2[:, :])
            nc.vector.tensor_scalar_add(sc_a[:, :], s2[:, :], 1.0)
            nc.vector.reciprocal(sc_b[:, :], sc_a[:, :])
            nc.scalar.activation(sc_c[:, :], s2[:, :], AFT.Sqrt,
                                 bias=eps_t[:, :])
            nc.vector.reciprocal(sc_d[:, :], sc_c[:, :])
            nc.vector.tensor_mul(sc_e[:, :], sc_b[:, :], sc_d[:, :])
            nc.vector.tensor_mul(sc_e[:, :], sc_e[:, :], s2[:, :])
            tgt = o_fin if it == ROUTINGS - 1 else o_acc
            if it == 1:
                nc.vector.tensor_scalar_mul(o_fin[:, :], sm[:, :], sc_e[:, :])
                nc.vector.tensor_add(o_acc[:, :], o_acc[:, :], o_fin[:, :])
            else:
                nc.vector.tensor_scalar_mul(tgt[:, :], sm[:, :], sc_e[:, :])
            if it == ROUTINGS - 1:
                nc.sync.dma_start(out=out_d[:, :], in_=o_fin[:, :])
                return
            # Obd_b (256,16 block-diag of O_b) as columns of o_acc.T halves
            tps = []
            for h, t_sb in ((0, t1_sb), (1, t2_sb)):
                tp = ps_bb.tile([128, 128], f32, tag="bb", name="tp",
                                padded_shape=[128, 512])
                nc.tensor.transpose(tp[:, :], o_acc[:, h * 128:(h + 1) * 128],
                                    ident_t[:, :])
                nc.vector.tensor_copy(t_sb[:, :], tp[:, :])
                tps.append(tp)
            # Wo_b = W @ Obd_b, accumulated over the two 128-row halves of W.T
            wo = ps_bb.tile([64, NB * K], f32, tag="bb", name="wo",
                            padded_shape=[64, 512])
            for h2 in range(2):
                for b in range(NB):
                    nc.tensor.matmul(
                        wo[:, b * K:(b + 1) * K],
                        lhsT=wt_t[:, h2 * D:(h2 + 1) * D],
                        rhs=(t1_sb, t2_sb)[h2][:, b * K:(b + 1) * K],
                        start=(h2 == 0 and b == 0),
                        stop=(h2 == 1 and b == NB - 1))
            for b in range(NB):
                p, h = b // 2, b % 2
                nc.vector.tensor_copy(
                    wop[p][64 * h:64 * h + 64, 16 * h:16 * h + 16],
                    wo[:, b * K:(b + 1) * K])

        for rep in range(reps):
            if "nodma" not in ablate:
                for q in range(nsubq):
                    nc.sync.dma_start(out=un_t[q][:, :], in_=un_d[q])
                for p in range(NP):
                    for q in range(nsub):
                        nc.sync.dma_start(
                            out=ut_t[p][q][:, :],
                            in_=ut_d[p, :, q * subcols:(q + 1) * subcols])
            elif rep == 0:
                for q in range(nsubq):
                    nc.vector.memset(un_t[q][:, 0:2], 0.0)
                for p in range(NP):
                    for q in range(nsub):
                        nc.vector.memset(ut_t[p][q][:, 0:2], 0.0)
            if "nocompute" not in ablate:
                for it in range(ROUTINGS):
                    routing_pass(it)
                    finalize(it)
            else:
                nc.vector.memset(o_fin[:, :], 0.0)
                nc.sync.dma_start(out=out_d[:, :], in_=o_fin[:, :])
            if rep < reps - 1:
                tc.strict_bb_all_engine_barrier()

    nc.compile()
    return nc


def host_inputs(u_shard, W):
    """Per-core DRAM inputs from an (8, N, 64) f32 batch shard + W (64, 256)."""
    n = u_shard.shape[1]
    ut = np.ascontiguousarray(
        u_shard.reshape(NP, 2, n, D).transpose(0, 1, 3, 2).reshape(NP, 128, n)
    ).astype(U_NP)
    un = np.ascontiguousarray(
        u_shard.reshape(NP, 2, n // 512, 4, CHUNK, D)
        .transpose(2, 4, 3, 0, 1, 5).reshape(n // 512, 128, 2048)
    ).astype(U_NP)
    return {"ut": ut, "un": un}


def host_consts(W):
    Wf = np.asarray(W, np.float32)
    wt = np.ascontiguousarray(Wf.T.reshape(2, 128, D)).astype(U_NP)
    wsb = np.ascontiguousarray(np.concatenate([Wf, Wf], 0)).astype(U_NP)
    base = np.kron(np.eye(K, dtype=np.float32), np.ones((1, DCAP), np.float32))
    mask = np.ascontiguousarray(np.tile(base, (NB, 1)))
    ident = np.eye(128, dtype=np.float32)
    return {"wt": wt, "wsb": wsb, "mask": mask, "ident": ident}


def extract_output(res_out):
    """(128, 256) masked f32 -> (8, 16, 16) squashed capsule outputs."""
    ar = np.arange(K)
    return res_out.reshape(NB, K, K, DCAP)[:, ar, ar, :]


_PROG_CACHE = {}


def _get_prog(n=N_FULL, reps=1):
    key = (n, reps)
    if key not in _PROG_CACHE:
        _PROG_CACHE[key] = build_program(n, reps)
    return _PROG_CACHE[key]


def kernel(u_vecs, W):
    u = np.ascontiguousarray(np.asarray(u_vecs, np.float32))
    assert u.shape == (B, N_FULL, D)
    nc = _get_prog()
    consts = host_consts(W)
    in_maps = [dict(consts, **host_inputs(u[c * NB:(c + 1) * NB], W))
               for c in range(NCORES)]
    res = run_bass_kernel_spmd(nc, in_maps, core_ids=list(range(NCORES)))
    return np.concatenate(
        [extract_output(res.results[c]["out"]) for c in range(NCORES)], axis=0
    ).astype(np.float32)
